# revision 12
# baseline (speedup 1.0000x reference)
"""Trainium2 Bass kernel for nn_Decoder (CSS sampled-softmax decoder loss).

Computation (see reference):
  en_rec_loss[b] = sum_s en_mask[b,s] * (zs[b,s]@W_en[x_en[b,s]] - ln(D_en[b,s]))
  fr_rec_loss[b] = sum_f fr_mask[b,f] * ln( sum_s exp(be_fr[b,f]@zs[b,s] - ln(D_fr[b,s])) )
  D[b,s] = sum_p exp(zs@pos_e[p]) + kappa * sum_n exp(zs@neg_e[n])

Sharding: data-parallel over batch. Each of the 8 cores gets B/8 = 8 batch
rows (512 tokens); the sampled embedding slices (pos+neg rows of each table,
gathered host-side, cast to bf16 and pre-transposed) are replicated to all
cores. No collectives.

Device kernel per core:
  - score matmuls  zT.T @ E_T  (bf16, K=256 as 2x128) into 2048-wide PSUM
    groups; ScalarE Exp with accum_out gives per-token partial sums; the
    kappa weight on negative samples is folded into the Exp bias (ln kappa)
    and zero-padding columns are corrected in the Ln bias.
  - en numerator via DVE tensor_tensor_reduce on fp32 token-major z/be.
  - fr alignment: per-batch 64x64 matmuls, Exp(score - lnD) via per-s bias,
    ones-matmul to reduce over s, Ln, mask, per-batch reduce.
  - per-batch sums of en contributions via a half-ones matmul.
"""

import os
from contextlib import ExitStack

import numpy as np

import concourse.bass as bass
import concourse.bacc as bacc
import concourse.tile as tile
from concourse import mybir
from concourse.bass_utils import run_bass_kernel_spmd

import ml_dtypes

BF16 = ml_dtypes.bfloat16

N_CORES = 8
B, S, D = 64, 64, 256
TOK = B * S                      # 4096 tokens
TOK_CORE = TOK // N_CORES        # 512 tokens per core
TOK_TILES = TOK_CORE // 128      # 4 token tiles per core
B_CORE = B // N_CORES            # 8 batch rows per core
CHUNK = 2048                     # score columns per PSUM group (4 banks f32)

# Results of the last traced run (for test harness use).
last_results = None

_nc_cache = {}


def _build_nc(npos_g_en, nneg_g_en, npos_g_fr, nneg_g_fr,
              lnk_en, lnk_fr, corr_en, corr_fr, phases="ABCD"):
    """Build the single-core SPMD Bass module.

    npos_g/nneg_g: number of 2048-wide column groups of positive / negative
    samples per language. lnk: ln(kappa) folded into the Exp bias of negative
    groups. corr: additive constant in the Ln bias correcting for zero-padded
    columns, i.e. ln(denom) = Ln(raw_sum + corr).
    """
    f32 = mybir.dt.float32
    bf16 = mybir.dt.bfloat16
    G_en = npos_g_en + nneg_g_en
    G_fr = npos_g_fr + nneg_g_fr
    C_en = G_en * CHUNK
    C_fr = G_fr * CHUNK

    nc = bacc.Bacc()

    zT = nc.dram_tensor("zT", [128, 2, TOK_CORE], bf16, kind="ExternalInput")
    ztok = nc.dram_tensor("ztok", [TOK_CORE, D], f32, kind="ExternalInput")
    betok = nc.dram_tensor("betok", [TOK_CORE, D], f32, kind="ExternalInput")
    befrT = nc.dram_tensor("befrT", [128, 2, TOK_CORE], bf16, kind="ExternalInput")
    Een = nc.dram_tensor("Een", [128, 2, C_en], bf16, kind="ExternalInput")
    Efr = nc.dram_tensor("Efr", [128, 2, C_fr], bf16, kind="ExternalInput")
    m_en = nc.dram_tensor("m_en", [TOK_CORE, 1], f32, kind="ExternalInput")
    m_fr = nc.dram_tensor("m_fr", [1, TOK_CORE], f32, kind="ExternalInput")
    o_en = nc.dram_tensor("o_en", [2, TOK_TILES], f32, kind="ExternalOutput")
    o_fr = nc.dram_tensor("o_fr", [1, B_CORE], f32, kind="ExternalOutput")

    AF = mybir.ActivationFunctionType
    AX = mybir.AxisListType
    OP = mybir.AluOpType

    with tile.TileContext(nc) as tc, ExitStack() as ctx:
        singles = ctx.enter_context(tc.tile_pool(name="singles", bufs=1))
        epool = ctx.enter_context(tc.tile_pool(name="epool", bufs=3))
        expool = ctx.enter_context(tc.tile_pool(name="expool", bufs=3))
        accpool = ctx.enter_context(tc.tile_pool(name="accpool", bufs=2 * TOK_TILES))
        tokpool = ctx.enter_context(tc.tile_pool(name="tokpool", bufs=2))
        smalls = ctx.enter_context(tc.tile_pool(name="smalls", bufs=4))

        # --- constants & whole-kernel resident tiles ---
        zT_s = singles.tile([128, 2, TOK_CORE], bf16)
        nc.sync.dma_start(zT_s, zT[:])
        befrT_s = singles.tile([128, 2, TOK_CORE], bf16)
        nc.sync.dma_start(befrT_s, befrT[:])

        halfones = singles.tile([128, 2], f32)
        nc.vector.memset(halfones, 0.0)
        nc.vector.memset(halfones[0:64, 0:1], 1.0)
        nc.vector.memset(halfones[64:128, 1:2], 1.0)
        ones64 = singles.tile([64, 1], f32)
        nc.vector.memset(ones64, 1.0)
        bias_lnk = {}
        bias_corr = {}
        for name, lnk, corr in (("en", lnk_en, corr_en), ("fr", lnk_fr, corr_fr)):
            t = singles.tile([128, 1], f32, name=f"bias_lnk_{name}", tag=f"bias_lnk_{name}")
            nc.vector.memset(t, float(lnk))
            bias_lnk[name] = t
            t = singles.tile([128, 1], f32, name=f"bias_corr_{name}", tag=f"bias_corr_{name}")
            nc.vector.memset(t, float(corr))
            bias_corr[name] = t

        # --- Phase A: exp-sum partials for both languages ---
        langs = [
            ("en", Een, G_en, npos_g_en, lnk_en),
            ("fr", Efr, G_fr, npos_g_fr, lnk_fr),
        ]
        acc = {}
        for name, _, G, _, _ in langs:
            for j in range(TOK_TILES):
                acc[name, j] = accpool.tile([128, G], f32, tag=f"acc_{name}",
                                            name=f"acc_{name}_{j}")

        with tc.tile_pool(name="psumA", bufs=2, space="PSUM") as psumA:
            for name, E_dram, G, npos_g, lnk in langs:
                for g in range(G):
                    Eg = epool.tile([128, 2, CHUNK], bf16, tag="Eg")
                    nc.sync.dma_start(Eg, E_dram[:, :, g * CHUNK:(g + 1) * CHUNK])
                    bias = 0.0 if g < npos_g else bias_lnk[name]
                    for j in range(TOK_TILES):
                        ps = psumA.tile([128, CHUNK], f32, tag="psA")
                        for c in range(2):
                            for nb in range(CHUNK // 512):
                                nc.tensor.matmul(
                                    ps[:, nb * 512:(nb + 1) * 512],
                                    zT_s[:, c, j * 128:(j + 1) * 128],
                                    Eg[:, c, nb * 512:(nb + 1) * 512],
                                    start=(c == 0),
                                    stop=(c == 1),
                                )
                        ex = expool.tile([128, CHUNK], bf16, tag="ex")
                        nc.scalar.activation(
                            ex, ps, AF.Exp, bias=bias,
                            accum_out=acc[name, j][:, g:g + 1],
                        )

            # --- Phase B: denominators, en contributions, fr -ln(D) ---
            contrib = singles.tile([128, TOK_TILES], f32)
            nld = singles.tile([128, TOK_TILES], f32)
            for name, _, G, _, _ in langs:
                for j in range(TOK_TILES):
                    draw = smalls.tile([128, 1], f32, tag="draw")
                    nc.vector.reduce_sum(draw, acc[name, j], axis=AX.X)
                    ld = smalls.tile([128, 1], f32, tag="ld")
                    nc.scalar.activation(ld, draw, AF.Ln, bias=bias_corr[name])
                    if name == "en":
                        zt = tokpool.tile([128, D], f32, tag="zt")
                        nc.sync.dma_start(zt, ztok[j * 128:(j + 1) * 128, :])
                        bt = tokpool.tile([128, D], f32, tag="bt")
                        nc.sync.dma_start(bt, betok[j * 128:(j + 1) * 128, :])
                        prod = tokpool.tile([128, D], f32, tag="prod")
                        num = smalls.tile([128, 1], f32, tag="num")
                        nc.vector.tensor_tensor(prod, zt, bt, OP.mult)
                        nc.vector.reduce_sum(num, prod, axis=AX.X)
                        mt = smalls.tile([128, 1], f32, tag="mt")
                        nc.sync.dma_start(mt, m_en[j * 128:(j + 1) * 128, :])
                        # contrib = (num - ln(D)) * mask
                        nc.vector.tensor_scalar(
                            out=contrib[:, j:j + 1], in0=num, scalar1=ld,
                            scalar2=mt, op0=OP.subtract, op1=OP.mult,
                        )
                    else:
                        nc.vector.tensor_scalar_mul(nld[:, j:j + 1], ld, -1.0)

        # rearrange fr -ln(D): nld[(h*64+s), j] -> nd[s, j, h]  (batch b = 2j+h)
        nd = singles.tile([64, TOK_TILES, 2], f32)
        nc.sync.dma_start(nd[:, :, 0], nld[0:64, :])
        nc.sync.dma_start(nd[:, :, 1], nld[64:128, :])

        with tc.tile_pool(name="psumB", bufs=2, space="PSUM") as psumB:
            fro = singles.tile([1, B_CORE], f32)
            if "C" in phases:
                # --- Phase C: fr alignment scores ---
                expall = singles.tile([64, B_CORE, S], f32)
                for b in range(B_CORE):
                    j, h = b // 2, b % 2
                    ps2 = psumB.tile([64, S], f32, tag="ps2")
                    for c in range(2):
                        nc.tensor.matmul(
                            ps2,
                            zT_s[:, c, b * 64:(b + 1) * 64],
                            befrT_s[:, c, b * 64:(b + 1) * 64],
                            start=(c == 0),
                            stop=(c == 1),
                        )
                    nc.scalar.activation(
                        expall[:, b, :], ps2, AF.Exp, bias=nd[:, j, h:h + 1],
                    )
                Tps = psumB.tile([1, B_CORE * S], f32, tag="Tps")
                nc.tensor.matmul(Tps, ones64, expall)
                lnT = singles.tile([1, B_CORE * S], f32)
                nc.scalar.activation(lnT, Tps, AF.Ln)
                mfr = singles.tile([1, B_CORE * S], f32)
                nc.sync.dma_start(mfr, m_fr[:])
                frc = singles.tile([1, B_CORE, S], f32)
                nc.vector.tensor_tensor(
                    frc.rearrange("p b s -> p (b s)"), lnT, mfr, OP.mult)
                nc.vector.reduce_sum(fro, frc, axis=AX.X)
            else:
                nc.vector.memset(fro, 0.0)
            nc.sync.dma_start(o_fr[:], fro)

            eno = singles.tile([2, TOK_TILES], f32)
            if "D" in phases:
                # --- Phase D: en per-batch sums ---
                enps = psumB.tile([2, TOK_TILES], f32, tag="enps")
                nc.tensor.matmul(enps, halfones, contrib)
                nc.vector.tensor_copy(eno, enps)
            else:
                nc.vector.tensor_copy(eno, contrib[0:2, :])
            nc.sync.dma_start(o_en[:], eno)

    nc.finalize()
    return nc


def _get_nc(key):
    if key not in _nc_cache:
        _nc_cache[key] = _build_nc(*key)
    return _nc_cache[key]


def _prep_lang(W, pos, neg, kappa):
    """Gather sampled rows, zero-pad each segment to a CHUNK multiple, and
    return the [128, 2, C] bf16 pre-transposed slice plus bias constants."""
    P = int(pos.shape[0])
    NNEG = int(neg.shape[0])
    npos_g = -(-P // CHUNK)
    nneg_g = -(-NNEG // CHUNK)
    Ppad = npos_g * CHUNK
    C = Ppad + nneg_g * CHUNK
    E = np.zeros((C, D), np.float32)
    E[:P] = W[pos]
    E[Ppad:Ppad + NNEG] = W[neg]
    # each zero pad column contributes exp(0 [+ ln kappa]) to the raw sum
    corr = -((Ppad - P) + kappa * (nneg_g * CHUNK - NNEG))
    ET = np.ascontiguousarray(
        E.T.reshape(2, 128, C).transpose(1, 0, 2)).astype(BF16)
    return ET, npos_g, nneg_g, float(np.log(kappa)), float(corr)


def _t128(a):
    """[T, D] -> [128, 2, T] (partition-major transposed, bf16)."""
    T = a.shape[0]
    return np.ascontiguousarray(
        a.T.reshape(2, 128, T).transpose(1, 0, 2)).astype(BF16)


def _prepare(inputs):
    """Host-side sharding prep: returns (nc, in_maps) for the 8 cores."""
    zs = np.asarray(inputs["zs"], np.float32)
    x_en = np.asarray(inputs["x_en"]).astype(np.int64)
    x_fr = np.asarray(inputs["x_fr"]).astype(np.int64)
    en_mask = np.asarray(inputs["en_mask"], np.float32)
    fr_mask = np.asarray(inputs["fr_mask"], np.float32)
    W_en = np.asarray(inputs["W_en"], np.float32)
    W_fr = np.asarray(inputs["W_fr"], np.float32)
    pos_en = np.asarray(inputs["pos_en"]).astype(np.int64)
    neg_en = np.asarray(inputs["neg_en"]).astype(np.int64)
    pos_fr = np.asarray(inputs["pos_fr"]).astype(np.int64)
    neg_fr = np.asarray(inputs["neg_fr"]).astype(np.int64)
    kappa_en = float(np.asarray(inputs["kappa_en"]))
    kappa_fr = float(np.asarray(inputs["kappa_fr"]))

    z = zs.reshape(TOK, D)
    ETen, npg_en, nng_en, lnk_en, corr_en = _prep_lang(W_en, pos_en, neg_en, kappa_en)
    ETfr, npg_fr, nng_fr, lnk_fr, corr_fr = _prep_lang(W_fr, pos_fr, neg_fr, kappa_fr)

    nc = _get_nc((npg_en, nng_en, npg_fr, nng_fr,
                  lnk_en, lnk_fr, corr_en, corr_fr))

    be_en = W_en[x_en.reshape(TOK)]
    be_fr = W_fr[x_fr.reshape(TOK)]
    men_flat = en_mask.reshape(TOK, 1).astype(np.float32)

    in_maps = []
    for k in range(N_CORES):
        t0, t1 = k * TOK_CORE, (k + 1) * TOK_CORE
        in_maps.append({
            "zT": _t128(z[t0:t1]),
            "ztok": np.ascontiguousarray(z[t0:t1]),
            "betok": np.ascontiguousarray(be_en[t0:t1]),
            "befrT": _t128(be_fr[t0:t1]),
            "Een": ETen,
            "Efr": ETfr,
            "m_en": np.ascontiguousarray(men_flat[t0:t1]),
            "m_fr": np.ascontiguousarray(
                fr_mask[k * B_CORE:(k + 1) * B_CORE].reshape(1, TOK_CORE)),
        })
    return nc, in_maps


def kernel(**inputs):
    global last_results

    nc, in_maps = _prepare(inputs)

    trace = bool(int(os.environ.get("KERNEL_TRACE", "0")))
    res = run_bass_kernel_spmd(nc, in_maps, core_ids=list(range(N_CORES)),
                               trace=trace)
    last_results = res

    en = np.empty(B, np.float32)
    fr = np.empty(B, np.float32)
    for k in range(N_CORES):
        en[k * B_CORE:(k + 1) * B_CORE] = res.results[k]["o_en"].T.reshape(B_CORE)
        fr[k * B_CORE:(k + 1) * B_CORE] = res.results[k]["o_fr"].reshape(B_CORE)
    return en, fr


# revision 14
# speedup vs baseline: 1.0350x; 1.0350x over previous
"""Trainium2 Bass kernel for nn_Decoder (CSS sampled-softmax decoder loss).

Computation (see reference):
  en_rec_loss[b] = sum_s en_mask[b,s] * (zs[b,s]@W_en[x_en[b,s]] - ln(D_en[b,s]))
  fr_rec_loss[b] = sum_f fr_mask[b,f] * ln( sum_s exp(be_fr[b,f]@zs[b,s] - ln(D_fr[b,s])) )
  D[b,s] = sum_p exp(zs@pos_e[p]) + kappa * sum_n exp(zs@neg_e[n])

Sharding: data-parallel over batch. Each of the 8 cores gets B/8 = 8 batch
rows (512 tokens); the sampled embedding slices (pos+neg rows of each table,
gathered host-side, cast to bf16 and pre-transposed) are replicated to all
cores. No collectives.

Device kernel per core:
  - score matmuls  zT.T @ E_T  (bf16, K=256 as 2x128) into 2048-wide PSUM
    groups; ScalarE Exp with accum_out gives per-token partial sums; the
    kappa weight on negative samples is folded into the Exp bias (ln kappa)
    and zero-padding columns are corrected in the Ln bias.
  - en numerator via DVE tensor_tensor_reduce on fp32 token-major z/be.
  - fr alignment: per-batch 64x64 matmuls, Exp(score - lnD) via per-s bias,
    ones-matmul to reduce over s, Ln, mask, per-batch reduce.
  - per-batch sums of en contributions via a half-ones matmul.
"""

import os
from contextlib import ExitStack

import numpy as np

import concourse.bass as bass
import concourse.bacc as bacc
import concourse.tile as tile
from concourse import mybir
from concourse.bass_utils import run_bass_kernel_spmd

import ml_dtypes

BF16 = ml_dtypes.bfloat16

N_CORES = 8
B, S, D = 64, 64, 256
TOK = B * S                      # 4096 tokens
TOK_CORE = TOK // N_CORES        # 512 tokens per core
TOK_TILES = TOK_CORE // 128      # 4 token tiles per core
B_CORE = B // N_CORES            # 8 batch rows per core
CHUNK = 2048                     # score columns per PSUM group (4 banks f32)

# Results of the last traced run (for test harness use).
last_results = None

_nc_cache = {}


def _build_nc(npos_g_en, nneg_g_en, npos_g_fr, nneg_g_fr,
              lnk_en, lnk_fr, corr_en, corr_fr):
    """Build the single-core SPMD Bass module.

    npos_g/nneg_g: number of 2048-wide column groups of positive / negative
    samples per language. lnk: ln(kappa) folded into the Exp bias of negative
    groups. corr: additive constant in the Ln bias correcting for zero-padded
    columns, i.e. ln(denom) = Ln(raw_sum + corr).
    """
    f32 = mybir.dt.float32
    bf16 = mybir.dt.bfloat16
    G_en = npos_g_en + nneg_g_en
    G_fr = npos_g_fr + nneg_g_fr
    C_en = G_en * CHUNK
    C_fr = G_fr * CHUNK

    nc = bacc.Bacc()

    zT = nc.dram_tensor("zT", [128, 2, TOK_CORE], bf16, kind="ExternalInput")
    ztok = nc.dram_tensor("ztok", [TOK_CORE, D], f32, kind="ExternalInput")
    betok = nc.dram_tensor("betok", [TOK_CORE, D], f32, kind="ExternalInput")
    befrT = nc.dram_tensor("befrT", [128, 2, TOK_CORE], bf16, kind="ExternalInput")
    Een = nc.dram_tensor("Een", [128, 2, C_en], bf16, kind="ExternalInput")
    Efr = nc.dram_tensor("Efr", [128, 2, C_fr], bf16, kind="ExternalInput")
    m_en = nc.dram_tensor("m_en", [TOK_CORE, 1], f32, kind="ExternalInput")
    m_fr = nc.dram_tensor("m_fr", [1, TOK_CORE], f32, kind="ExternalInput")
    o_en = nc.dram_tensor("o_en", [2, TOK_TILES], f32, kind="ExternalOutput")
    o_fr = nc.dram_tensor("o_fr", [1, B_CORE], f32, kind="ExternalOutput")

    AF = mybir.ActivationFunctionType
    AX = mybir.AxisListType
    OP = mybir.AluOpType

    with tile.TileContext(nc) as tc, ExitStack() as ctx:
        singles = ctx.enter_context(tc.tile_pool(name="singles", bufs=1))
        epool = ctx.enter_context(tc.tile_pool(name="epool", bufs=4))
        expool = ctx.enter_context(tc.tile_pool(name="expool", bufs=3))
        accpool = ctx.enter_context(tc.tile_pool(name="accpool", bufs=2 * TOK_TILES))
        tokpool = ctx.enter_context(tc.tile_pool(name="tokpool", bufs=2))
        smalls = ctx.enter_context(tc.tile_pool(name="smalls", bufs=4))

        langs = [
            ("en", Een, G_en, npos_g_en, lnk_en),
            ("fr", Efr, G_fr, npos_g_fr, lnk_fr),
        ]

        # --- prefetch first embedding group, then resident tiles ---
        Eg_first = epool.tile([128, 2, CHUNK], bf16, tag="Eg", name="Eg_first")
        nc.sync.dma_start(Eg_first, langs[0][1][:, :, 0:CHUNK])
        zT_s = singles.tile([128, 2, TOK_CORE], bf16)
        nc.sync.dma_start(zT_s, zT[:])
        befrT_s = singles.tile([128, 2, TOK_CORE], bf16)
        nc.sync.dma_start(befrT_s, befrT[:])

        halfones = singles.tile([128, 2], f32)
        nc.vector.memset(halfones, 0.0)
        nc.vector.memset(halfones[0:64, 0:1], 1.0)
        nc.vector.memset(halfones[64:128, 1:2], 1.0)
        ones128 = singles.tile([128, 1], f32)
        nc.vector.memset(ones128, 1.0)
        bias_lnk = {}
        bias_corr = {}
        for name, lnk, corr in (("en", lnk_en, corr_en), ("fr", lnk_fr, corr_fr)):
            t = singles.tile([128, 1], f32, name=f"bias_lnk_{name}", tag=f"bias_lnk_{name}")
            nc.vector.memset(t, float(lnk))
            bias_lnk[name] = t
            t = singles.tile([128, 1], f32, name=f"bias_corr_{name}", tag=f"bias_corr_{name}")
            nc.vector.memset(t, float(corr))
            bias_corr[name] = t

        # --- en numerators (independent of the score stream; runs early) ---
        num_buf = singles.tile([128, TOK_TILES], f32)
        for j in range(TOK_TILES):
            zt = tokpool.tile([128, D], f32, tag="zt")
            nc.sync.dma_start(zt, ztok[j * 128:(j + 1) * 128, :])
            bt = tokpool.tile([128, D], f32, tag="bt")
            nc.sync.dma_start(bt, betok[j * 128:(j + 1) * 128, :])
            prod = tokpool.tile([128, D], f32, tag="prod")
            nc.vector.tensor_tensor(prod, zt, bt, OP.mult)
            nc.vector.reduce_sum(num_buf[:, j:j + 1], prod, axis=AX.X)

        # fr raw-exp alignment matrix [s, (b, f)]; rows 64:128 zeroed so the
        # column-sum matmul can contract over a full 128 partitions.
        expall = singles.tile([128, B_CORE, S], f32)
        nc.vector.memset(expall[64:128], 0.0)

        acc = {}
        for name, _, G, _, _ in langs:
            for j in range(TOK_TILES):
                acc[name, j] = accpool.tile([128, G], f32, tag=f"acc_{name}",
                                            name=f"acc_{name}_{j}")

        with tc.tile_pool(name="psumA", bufs=2, space="PSUM") as psumA:
            # --- Phase A: exp-sum partials for both languages ---
            for li, (name, E_dram, G, npos_g, lnk) in enumerate(langs):
                for g in range(G):
                    if li == 0 and g == 0:
                        Eg = Eg_first
                    else:
                        Eg = epool.tile([128, 2, CHUNK], bf16, tag="Eg")
                        nc.sync.dma_start(Eg, E_dram[:, :, g * CHUNK:(g + 1) * CHUNK])
                    bias = 0.0 if g < npos_g else bias_lnk[name]
                    for j in range(TOK_TILES):
                        ps = psumA.tile([128, CHUNK], f32, tag="psA")
                        for c in range(2):
                            for nb in range(CHUNK // 512):
                                nc.tensor.matmul(
                                    ps[:, nb * 512:(nb + 1) * 512],
                                    zT_s[:, c, j * 128:(j + 1) * 128],
                                    Eg[:, c, nb * 512:(nb + 1) * 512],
                                    start=(c == 0),
                                    stop=(c == 1),
                                )
                        ex = expool.tile([128, CHUNK], bf16, tag="ex")
                        nc.scalar.activation(
                            ex, ps, AF.Exp, bias=bias,
                            accum_out=acc[name, j][:, g:g + 1],
                        )

            # --- Phase C1: fr alignment scores, raw exp (joins the A stream) ---
            psC = psumA.tile([128, CHUNK], f32, tag="psA", name="psC")
            for b in range(B_CORE):
                for c in range(2):
                    nc.tensor.matmul(
                        psC[0:64, b * 64:(b + 1) * 64],
                        zT_s[:, c, b * 64:(b + 1) * 64],
                        befrT_s[:, c, b * 64:(b + 1) * 64],
                        start=(c == 0),
                        stop=(c == 1),
                    )
            nc.scalar.activation(
                expall[0:64].rearrange("p b s -> p (b s)"),
                psC[0:64, 0:B_CORE * S], AF.Exp)

            # --- Phase B: denominators -> en contribs + fr 1/D ---
            contrib = singles.tile([128, TOK_TILES], f32)
            iD = singles.tile([128, TOK_TILES], f32)
            for name, _, G, _, _ in langs:
                for j in range(TOK_TILES):
                    draw = smalls.tile([128, 1], f32, tag="draw")
                    nc.vector.reduce_sum(draw, acc[name, j], axis=AX.X)
                    if name == "en":
                        ld = smalls.tile([128, 1], f32, tag="ld")
                        nc.scalar.activation(ld, draw, AF.Ln, bias=bias_corr[name])
                        mt = smalls.tile([128, 1], f32, tag="mt")
                        nc.sync.dma_start(mt, m_en[j * 128:(j + 1) * 128, :])
                        # contrib = (num - ln(D)) * mask
                        nc.vector.tensor_scalar(
                            out=contrib[:, j:j + 1], in0=num_buf[:, j:j + 1],
                            scalar1=ld, scalar2=mt, op0=OP.subtract, op1=OP.mult,
                        )
                    else:
                        dfull = smalls.tile([128, 1], f32, tag="dfull")
                        nc.vector.tensor_scalar_add(dfull, draw, bias_corr[name])
                        nc.vector.reciprocal(iD[:, j:j + 1], dfull)

        # rearrange fr 1/D: iD[(h*64+s), j] -> nd[s, j, h]  (batch b = 2j+h)
        nd = singles.tile([64, TOK_TILES, 2], f32)
        nc.sync.dma_start(nd[:, :, 0], iD[0:64, :])
        nc.sync.dma_start(nd[:, :, 1], iD[64:128, :])

        with tc.tile_pool(name="psumB", bufs=2, space="PSUM") as psumB:
            # --- Phase C2: T[b,f] = sum_s exp * (1/D)[b,s]; then ln, mask ---
            for b in range(B_CORE):
                j, h = b // 2, b % 2
                nc.vector.tensor_scalar_mul(
                    expall[0:64, b, :], expall[0:64, b, :], nd[:, j, h:h + 1])
            Tps = psumB.tile([1, B_CORE * S], f32, tag="Tps")
            nc.tensor.matmul(Tps, ones128,
                             expall.rearrange("p b s -> p (b s)"))
            lnT = singles.tile([1, B_CORE * S], f32)
            nc.scalar.activation(lnT, Tps, AF.Ln)
            mfr = singles.tile([1, B_CORE * S], f32)
            nc.sync.dma_start(mfr, m_fr[:])
            frc = singles.tile([1, B_CORE, S], f32)
            nc.vector.tensor_tensor(
                frc.rearrange("p b s -> p (b s)"), lnT, mfr, OP.mult)
            fro = singles.tile([1, B_CORE], f32)
            nc.vector.reduce_sum(fro, frc, axis=AX.X)
            nc.sync.dma_start(o_fr[:], fro)

            # --- Phase D: en per-batch sums ---
            enps = psumB.tile([2, TOK_TILES], f32, tag="enps")
            nc.tensor.matmul(enps, halfones, contrib)
            eno = singles.tile([2, TOK_TILES], f32)
            nc.vector.tensor_copy(eno, enps)
            nc.sync.dma_start(o_en[:], eno)

    nc.finalize()
    return nc


def _get_nc(key):
    if key not in _nc_cache:
        _nc_cache[key] = _build_nc(*key)
    return _nc_cache[key]


def _prep_lang(W, pos, neg, kappa):
    """Gather sampled rows, zero-pad each segment to a CHUNK multiple, and
    return the [128, 2, C] bf16 pre-transposed slice plus bias constants."""
    P = int(pos.shape[0])
    NNEG = int(neg.shape[0])
    npos_g = -(-P // CHUNK)
    nneg_g = -(-NNEG // CHUNK)
    Ppad = npos_g * CHUNK
    C = Ppad + nneg_g * CHUNK
    E = np.zeros((C, D), np.float32)
    E[:P] = W[pos]
    E[Ppad:Ppad + NNEG] = W[neg]
    # each zero pad column contributes exp(0 [+ ln kappa]) to the raw sum
    corr = -((Ppad - P) + kappa * (nneg_g * CHUNK - NNEG))
    ET = np.ascontiguousarray(
        E.T.reshape(2, 128, C).transpose(1, 0, 2)).astype(BF16)
    return ET, npos_g, nneg_g, float(np.log(kappa)), float(corr)


def _t128(a):
    """[T, D] -> [128, 2, T] (partition-major transposed, bf16)."""
    T = a.shape[0]
    return np.ascontiguousarray(
        a.T.reshape(2, 128, T).transpose(1, 0, 2)).astype(BF16)


def _prepare(inputs):
    """Host-side sharding prep: returns (nc, in_maps) for the 8 cores."""
    zs = np.asarray(inputs["zs"], np.float32)
    x_en = np.asarray(inputs["x_en"]).astype(np.int64)
    x_fr = np.asarray(inputs["x_fr"]).astype(np.int64)
    en_mask = np.asarray(inputs["en_mask"], np.float32)
    fr_mask = np.asarray(inputs["fr_mask"], np.float32)
    W_en = np.asarray(inputs["W_en"], np.float32)
    W_fr = np.asarray(inputs["W_fr"], np.float32)
    pos_en = np.asarray(inputs["pos_en"]).astype(np.int64)
    neg_en = np.asarray(inputs["neg_en"]).astype(np.int64)
    pos_fr = np.asarray(inputs["pos_fr"]).astype(np.int64)
    neg_fr = np.asarray(inputs["neg_fr"]).astype(np.int64)
    kappa_en = float(np.asarray(inputs["kappa_en"]))
    kappa_fr = float(np.asarray(inputs["kappa_fr"]))

    z = zs.reshape(TOK, D)
    ETen, npg_en, nng_en, lnk_en, corr_en = _prep_lang(W_en, pos_en, neg_en, kappa_en)
    ETfr, npg_fr, nng_fr, lnk_fr, corr_fr = _prep_lang(W_fr, pos_fr, neg_fr, kappa_fr)

    nc = _get_nc((npg_en, nng_en, npg_fr, nng_fr,
                  lnk_en, lnk_fr, corr_en, corr_fr))

    be_en = W_en[x_en.reshape(TOK)]
    be_fr = W_fr[x_fr.reshape(TOK)]
    men_flat = en_mask.reshape(TOK, 1).astype(np.float32)

    in_maps = []
    for k in range(N_CORES):
        t0, t1 = k * TOK_CORE, (k + 1) * TOK_CORE
        in_maps.append({
            "zT": _t128(z[t0:t1]),
            "ztok": np.ascontiguousarray(z[t0:t1]),
            "betok": np.ascontiguousarray(be_en[t0:t1]),
            "befrT": _t128(be_fr[t0:t1]),
            "Een": ETen,
            "Efr": ETfr,
            "m_en": np.ascontiguousarray(men_flat[t0:t1]),
            "m_fr": np.ascontiguousarray(
                fr_mask[k * B_CORE:(k + 1) * B_CORE].reshape(1, TOK_CORE)),
        })
    return nc, in_maps


def kernel(**inputs):
    global last_results

    nc, in_maps = _prepare(inputs)

    trace = bool(int(os.environ.get("KERNEL_TRACE", "0")))
    res = run_bass_kernel_spmd(nc, in_maps, core_ids=list(range(N_CORES)),
                               trace=trace)
    last_results = res

    en = np.empty(B, np.float32)
    fr = np.empty(B, np.float32)
    for k in range(N_CORES):
        en[k * B_CORE:(k + 1) * B_CORE] = res.results[k]["o_en"].T.reshape(B_CORE)
        fr[k * B_CORE:(k + 1) * B_CORE] = res.results[k]["o_fr"].reshape(B_CORE)
    return en, fr


# revision 16
# speedup vs baseline: 1.0544x; 1.0187x over previous
"""Trainium2 Bass kernel for nn_Decoder (CSS sampled-softmax decoder loss).

Computation (see reference):
  en_rec_loss[b] = sum_s en_mask[b,s] * (zs[b,s]@W_en[x_en[b,s]] - ln(D_en[b,s]))
  fr_rec_loss[b] = sum_f fr_mask[b,f] * ln( sum_s exp(be_fr[b,f]@zs[b,s] - ln(D_fr[b,s])) )
  D[b,s] = sum_p exp(zs@pos_e[p]) + kappa * sum_n exp(zs@neg_e[n])

Sharding: data-parallel over batch. Each of the 8 cores gets B/8 = 8 batch
rows (512 tokens); the sampled embedding slices (pos+neg rows of each table,
gathered host-side, cast to bf16 and pre-transposed) are replicated to all
cores. No collectives.

Device kernel per core:
  - score matmuls  zT.T @ E_T  (bf16, K=256 as 2x128) into 2048-wide PSUM
    groups; ScalarE Exp with accum_out gives per-token partial sums; the
    kappa weight on negative samples is folded into the Exp bias (ln kappa)
    and zero-padding columns are corrected in the Ln bias.
  - en numerator via DVE tensor_tensor_reduce on fp32 token-major z/be.
  - fr alignment: per-batch 64x64 matmuls, Exp(score - lnD) via per-s bias,
    ones-matmul to reduce over s, Ln, mask, per-batch reduce.
  - per-batch sums of en contributions via a half-ones matmul.
"""

import os
from contextlib import ExitStack

import numpy as np

import concourse.bass as bass
import concourse.bacc as bacc
import concourse.tile as tile
from concourse import mybir
from concourse.bass_utils import run_bass_kernel_spmd

import ml_dtypes

BF16 = ml_dtypes.bfloat16

N_CORES = 8
B, S, D = 64, 64, 256
TOK = B * S                      # 4096 tokens
TOK_CORE = TOK // N_CORES        # 512 tokens per core
TOK_TILES = TOK_CORE // 128      # 4 token tiles per core
B_CORE = B // N_CORES            # 8 batch rows per core
CHUNK = 2048                     # score columns per PSUM group (4 banks f32)

# Results of the last traced run (for test harness use).
last_results = None

_nc_cache = {}


def _build_nc(npos_g_en, nneg_g_en, npos_g_fr, nneg_g_fr,
              lnk_en, lnk_fr, corr_en, corr_fr):
    """Build the single-core SPMD Bass module.

    npos_g/nneg_g: number of 2048-wide column groups of positive / negative
    samples per language. lnk: ln(kappa) folded into the Exp bias of negative
    groups. corr: additive constant in the Ln bias correcting for zero-padded
    columns, i.e. ln(denom) = Ln(raw_sum + corr).
    """
    f32 = mybir.dt.float32
    bf16 = mybir.dt.bfloat16
    G_en = npos_g_en + nneg_g_en
    G_fr = npos_g_fr + nneg_g_fr
    C_en = G_en * CHUNK
    C_fr = G_fr * CHUNK

    nc = bacc.Bacc()

    zT = nc.dram_tensor("zT", [128, 2, TOK_CORE], bf16, kind="ExternalInput")
    ztok = nc.dram_tensor("ztok", [TOK_CORE, D], f32, kind="ExternalInput")
    betok = nc.dram_tensor("betok", [TOK_CORE, D], f32, kind="ExternalInput")
    befrT = nc.dram_tensor("befrT", [128, 2, TOK_CORE], bf16, kind="ExternalInput")
    Een = nc.dram_tensor("Een", [128, 2, C_en], bf16, kind="ExternalInput")
    Efr = nc.dram_tensor("Efr", [128, 2, C_fr], bf16, kind="ExternalInput")
    m_en = nc.dram_tensor("m_en", [TOK_CORE, 1], f32, kind="ExternalInput")
    m_fr = nc.dram_tensor("m_fr", [1, TOK_CORE], f32, kind="ExternalInput")
    o_en = nc.dram_tensor("o_en", [2, TOK_TILES], f32, kind="ExternalOutput")
    o_fr = nc.dram_tensor("o_fr", [1, B_CORE], f32, kind="ExternalOutput")

    AF = mybir.ActivationFunctionType
    AX = mybir.AxisListType
    OP = mybir.AluOpType

    with tile.TileContext(nc) as tc, ExitStack() as ctx:
        singles = ctx.enter_context(tc.tile_pool(name="singles", bufs=1))
        epool = ctx.enter_context(tc.tile_pool(name="epool", bufs=4))
        expool = ctx.enter_context(tc.tile_pool(name="expool", bufs=3))
        accpool = ctx.enter_context(tc.tile_pool(name="accpool", bufs=2 * TOK_TILES))
        tokpool = ctx.enter_context(tc.tile_pool(name="tokpool", bufs=2))
        smalls = ctx.enter_context(tc.tile_pool(name="smalls", bufs=4))

        langs = [
            ("fr", Efr, G_fr, npos_g_fr, lnk_fr),
            ("en", Een, G_en, npos_g_en, lnk_en),
        ]

        # --- prefetch first embedding group (split over two queues), then
        # resident tiles on other engines' DGE queues to parallelize the ramp ---
        zT_s = singles.tile([128, 2, TOK_CORE], bf16)
        nc.scalar.dma_start(zT_s, zT[:])
        befrT_s = singles.tile([128, 2, TOK_CORE], bf16)
        nc.gpsimd.dma_start(befrT_s, befrT[:])
        Eg_first = epool.tile([128, 2, CHUNK], bf16, tag="Eg", name="Eg_first")
        nc.sync.dma_start(Eg_first[:, :, 0:CHUNK // 2],
                          langs[0][1][:, :, 0:CHUNK // 2])
        nc.gpsimd.dma_start(Eg_first[:, :, CHUNK // 2:CHUNK],
                            langs[0][1][:, :, CHUNK // 2:CHUNK])

        halfones = singles.tile([128, 2], f32)
        nc.vector.memset(halfones, 0.0)
        nc.vector.memset(halfones[0:64, 0:1], 1.0)
        nc.vector.memset(halfones[64:128, 1:2], 1.0)
        ones128 = singles.tile([128, 1], f32)
        nc.vector.memset(ones128, 1.0)
        bias_lnk = {}
        bias_corr = {}
        for name, lnk, corr in (("en", lnk_en, corr_en), ("fr", lnk_fr, corr_fr)):
            t = singles.tile([128, 1], f32, name=f"bias_lnk_{name}", tag=f"bias_lnk_{name}")
            nc.vector.memset(t, float(lnk))
            bias_lnk[name] = t
            t = singles.tile([128, 1], f32, name=f"bias_corr_{name}", tag=f"bias_corr_{name}")
            nc.vector.memset(t, float(corr))
            bias_corr[name] = t

        # fr raw-exp alignment matrix [s, (b, f)]; rows 64:128 zeroed so the
        # column-sum matmul can contract over a full 128 partitions.
        expall = singles.tile([128, B_CORE, S], f32)
        nc.vector.memset(expall[64:128], 0.0)

        acc = {}
        for name, _, G, _, _ in langs:
            for j in range(TOK_TILES):
                acc[name, j] = accpool.tile([128, G], f32, tag=f"acc_{name}",
                                            name=f"acc_{name}_{j}")

        with tc.tile_pool(name="psumA", bufs=2, space="PSUM") as psumA:
            # --- Phase C1: fr alignment scores, raw exp (first in the stream) ---
            psC = psumA.tile([128, CHUNK], f32, tag="psA", name="psC")
            for b in range(B_CORE):
                for c in range(2):
                    nc.tensor.matmul(
                        psC[0:64, b * 64:(b + 1) * 64],
                        zT_s[:, c, b * 64:(b + 1) * 64],
                        befrT_s[:, c, b * 64:(b + 1) * 64],
                        start=(c == 0),
                        stop=(c == 1),
                    )
            nc.scalar.activation(
                expall[0:64].rearrange("p b s -> p (b s)"),
                psC[0:64, 0:B_CORE * S], AF.Exp)

            # --- Phase A: exp-sum partials for both languages ---
            for li, (name, E_dram, G, npos_g, lnk) in enumerate(langs):
                for g in range(G):
                    if li == 0 and g == 0:
                        Eg = Eg_first
                    else:
                        Eg = epool.tile([128, 2, CHUNK], bf16, tag="Eg")
                        nc.sync.dma_start(Eg, E_dram[:, :, g * CHUNK:(g + 1) * CHUNK])
                    bias = 0.0 if g < npos_g else bias_lnk[name]
                    for j in range(TOK_TILES):
                        ps = psumA.tile([128, CHUNK], f32, tag="psA")
                        for c in range(2):
                            for nb in range(CHUNK // 512):
                                nc.tensor.matmul(
                                    ps[:, nb * 512:(nb + 1) * 512],
                                    zT_s[:, c, j * 128:(j + 1) * 128],
                                    Eg[:, c, nb * 512:(nb + 1) * 512],
                                    start=(c == 0),
                                    stop=(c == 1),
                                )
                        ex = expool.tile([128, CHUNK], bf16, tag="ex")
                        nc.scalar.activation(
                            ex, ps, AF.Exp, bias=bias,
                            accum_out=acc[name, j][:, g:g + 1],
                        )

            # --- en numerators (DVE; DMAs on gpsimd queue) ---
            num_buf = singles.tile([128, TOK_TILES], f32)
            for j in range(TOK_TILES):
                zt = tokpool.tile([128, D], f32, tag="zt")
                nc.gpsimd.dma_start(zt, ztok[j * 128:(j + 1) * 128, :])
                bt = tokpool.tile([128, D], f32, tag="bt")
                nc.gpsimd.dma_start(bt, betok[j * 128:(j + 1) * 128, :])
                prod = tokpool.tile([128, D], f32, tag="prod")
                nc.vector.tensor_tensor(prod, zt, bt, OP.mult)
                nc.vector.reduce_sum(num_buf[:, j:j + 1], prod, axis=AX.X)

            # --- Phase B: denominators -> en contribs + fr 1/D ---
            contrib = singles.tile([128, TOK_TILES], f32)
            iD = singles.tile([128, TOK_TILES], f32)
            for name, _, G, _, _ in langs:
                for j in range(TOK_TILES):
                    draw = smalls.tile([128, 1], f32, tag="draw")
                    nc.vector.reduce_sum(draw, acc[name, j], axis=AX.X)
                    if name == "en":
                        ld = smalls.tile([128, 1], f32, tag="ld")
                        nc.scalar.activation(ld, draw, AF.Ln, bias=bias_corr[name])
                        mt = smalls.tile([128, 1], f32, tag="mt")
                        nc.gpsimd.dma_start(mt, m_en[j * 128:(j + 1) * 128, :])
                        # contrib = (num - ln(D)) * mask
                        nc.vector.tensor_scalar(
                            out=contrib[:, j:j + 1], in0=num_buf[:, j:j + 1],
                            scalar1=ld, scalar2=mt, op0=OP.subtract, op1=OP.mult,
                        )
                    else:
                        dfull = smalls.tile([128, 1], f32, tag="dfull")
                        nc.vector.tensor_scalar_add(dfull, draw, bias_corr[name])
                        nc.vector.reciprocal(iD[:, j:j + 1], dfull)

        # rearrange fr 1/D: iD[(h*64+s), j] -> nd[s, j, h]  (batch b = 2j+h)
        nd = singles.tile([64, TOK_TILES, 2], f32)
        nc.gpsimd.dma_start(nd[:, :, 0], iD[0:64, :])
        nc.gpsimd.dma_start(nd[:, :, 1], iD[64:128, :])

        with tc.tile_pool(name="psumB", bufs=2, space="PSUM") as psumB:
            # --- Phase C2: T[b,f] = sum_s exp * (1/D)[b,s]; then ln, mask ---
            for b in range(B_CORE):
                j, h = b // 2, b % 2
                nc.vector.tensor_scalar_mul(
                    expall[0:64, b, :], expall[0:64, b, :], nd[:, j, h:h + 1])
            Tps = psumB.tile([1, B_CORE * S], f32, tag="Tps")
            nc.tensor.matmul(Tps, ones128,
                             expall.rearrange("p b s -> p (b s)"))
            lnT = singles.tile([1, B_CORE * S], f32)
            nc.scalar.activation(lnT, Tps, AF.Ln)
            mfr = singles.tile([1, B_CORE * S], f32)
            nc.gpsimd.dma_start(mfr, m_fr[:])
            frc = singles.tile([1, B_CORE, S], f32)
            nc.vector.tensor_tensor(
                frc.rearrange("p b s -> p (b s)"), lnT, mfr, OP.mult)
            fro = singles.tile([1, B_CORE], f32)
            nc.vector.reduce_sum(fro, frc, axis=AX.X)
            nc.sync.dma_start(o_fr[:], fro)

            # --- Phase D: en per-batch sums ---
            enps = psumB.tile([2, TOK_TILES], f32, tag="enps")
            nc.tensor.matmul(enps, halfones, contrib)
            eno = singles.tile([2, TOK_TILES], f32)
            nc.vector.tensor_copy(eno, enps)
            nc.sync.dma_start(o_en[:], eno)

    nc.finalize()
    return nc


def _get_nc(key):
    if key not in _nc_cache:
        _nc_cache[key] = _build_nc(*key)
    return _nc_cache[key]


def _prep_lang(W, pos, neg, kappa):
    """Gather sampled rows, zero-pad each segment to a CHUNK multiple, and
    return the [128, 2, C] bf16 pre-transposed slice plus bias constants."""
    P = int(pos.shape[0])
    NNEG = int(neg.shape[0])
    npos_g = -(-P // CHUNK)
    nneg_g = -(-NNEG // CHUNK)
    Ppad = npos_g * CHUNK
    C = Ppad + nneg_g * CHUNK
    E = np.zeros((C, D), np.float32)
    E[:P] = W[pos]
    E[Ppad:Ppad + NNEG] = W[neg]
    # each zero pad column contributes exp(0 [+ ln kappa]) to the raw sum
    corr = -((Ppad - P) + kappa * (nneg_g * CHUNK - NNEG))
    ET = np.ascontiguousarray(
        E.T.reshape(2, 128, C).transpose(1, 0, 2)).astype(BF16)
    return ET, npos_g, nneg_g, float(np.log(kappa)), float(corr)


def _t128(a):
    """[T, D] -> [128, 2, T] (partition-major transposed, bf16)."""
    T = a.shape[0]
    return np.ascontiguousarray(
        a.T.reshape(2, 128, T).transpose(1, 0, 2)).astype(BF16)


def _prepare(inputs):
    """Host-side sharding prep: returns (nc, in_maps) for the 8 cores."""
    zs = np.asarray(inputs["zs"], np.float32)
    x_en = np.asarray(inputs["x_en"]).astype(np.int64)
    x_fr = np.asarray(inputs["x_fr"]).astype(np.int64)
    en_mask = np.asarray(inputs["en_mask"], np.float32)
    fr_mask = np.asarray(inputs["fr_mask"], np.float32)
    W_en = np.asarray(inputs["W_en"], np.float32)
    W_fr = np.asarray(inputs["W_fr"], np.float32)
    pos_en = np.asarray(inputs["pos_en"]).astype(np.int64)
    neg_en = np.asarray(inputs["neg_en"]).astype(np.int64)
    pos_fr = np.asarray(inputs["pos_fr"]).astype(np.int64)
    neg_fr = np.asarray(inputs["neg_fr"]).astype(np.int64)
    kappa_en = float(np.asarray(inputs["kappa_en"]))
    kappa_fr = float(np.asarray(inputs["kappa_fr"]))

    z = zs.reshape(TOK, D)
    ETen, npg_en, nng_en, lnk_en, corr_en = _prep_lang(W_en, pos_en, neg_en, kappa_en)
    ETfr, npg_fr, nng_fr, lnk_fr, corr_fr = _prep_lang(W_fr, pos_fr, neg_fr, kappa_fr)

    nc = _get_nc((npg_en, nng_en, npg_fr, nng_fr,
                  lnk_en, lnk_fr, corr_en, corr_fr))

    be_en = W_en[x_en.reshape(TOK)]
    be_fr = W_fr[x_fr.reshape(TOK)]
    men_flat = en_mask.reshape(TOK, 1).astype(np.float32)

    in_maps = []
    for k in range(N_CORES):
        t0, t1 = k * TOK_CORE, (k + 1) * TOK_CORE
        in_maps.append({
            "zT": _t128(z[t0:t1]),
            "ztok": np.ascontiguousarray(z[t0:t1]),
            "betok": np.ascontiguousarray(be_en[t0:t1]),
            "befrT": _t128(be_fr[t0:t1]),
            "Een": ETen,
            "Efr": ETfr,
            "m_en": np.ascontiguousarray(men_flat[t0:t1]),
            "m_fr": np.ascontiguousarray(
                fr_mask[k * B_CORE:(k + 1) * B_CORE].reshape(1, TOK_CORE)),
        })
    return nc, in_maps


def kernel(**inputs):
    global last_results

    nc, in_maps = _prepare(inputs)

    trace = bool(int(os.environ.get("KERNEL_TRACE", "0")))
    res = run_bass_kernel_spmd(nc, in_maps, core_ids=list(range(N_CORES)),
                               trace=trace)
    last_results = res

    en = np.empty(B, np.float32)
    fr = np.empty(B, np.float32)
    for k in range(N_CORES):
        en[k * B_CORE:(k + 1) * B_CORE] = res.results[k]["o_en"].T.reshape(B_CORE)
        fr[k * B_CORE:(k + 1) * B_CORE] = res.results[k]["o_fr"].reshape(B_CORE)
    return en, fr
